# revision 14
# baseline (speedup 1.0000x reference)
import os
import numpy as np

B, T, H, L = 64, 2048, 256, 16
NCORES = 8
BS = B // NCORES          # 8 sequences per core
PTS = BS * T              # 16384 columns per core
S = 256                   # segments per sequence
TSEG = T // S             # 16 steps per segment
CSHIFT = 3.3              # exp shift keeping scaled-exp products near 1
NG = 8                    # one column group per sequence
GW = PTS // NG            # 2048 columns per group

LAST_EXEC_NS = None


def _build_nc(with_emissions=True, with_scan=True, with_emcopy=True,
              with_emout=True):
    import concourse.bass as bass
    import concourse.mybir as mybir
    from concourse.tile import TileContext

    f32 = mybir.dt.float32
    bf16 = mybir.dt.bfloat16
    fp8 = mybir.dt.float8e4
    EXP = mybir.ActivationFunctionType.Exp
    COPY = mybir.ActivationFunctionType.Copy

    nc = bass.Bass()

    xt = nc.dram_tensor("xt", [128, 2 * PTS], fp8, kind="ExternalInput")
    wtp = nc.dram_tensor("wtp", [128, 2 * L], fp8, kind="ExternalInput")
    cpack = nc.dram_tensor("cpack", [128, 258], f32, kind="ExternalInput")
    em_out = nc.dram_tensor("em_out", [128, T], bf16, kind="ExternalOutput")
    uy_out = nc.dram_tensor("uy_out", [128, 2 * S], f32,
                            kind="ExternalOutput")

    with TileContext(nc) as tc:
        with tc.tile_pool(name="singles", bufs=1) as singles:
            wts = singles.tile([128, 2 * L], fp8, tag="wts")
            cps = singles.tile([128, 258], f32, tag="cps")
            expTDs = singles.tile([128, 128], bf16, tag="expTD")
            expTTDs = singles.tile([128, 128], bf16, tag="expTTD")
            cbias = singles.tile([128, 1], f32, tag="cbias")
            wt0s = wts[:, 0:L]
            wt1s = wts[:, L:2 * L]
            acss = cps[:, 256:257]
            ests = cps[:, 257:258]
            em128 = singles.tile([128, T], bf16, tag="em128")
            eem = singles.tile([128, T], bf16, tag="eem")

            nc.vector.memset(cbias, -CSHIFT)
            nc.sync.dma_start(wt0s, wt0[:, :])
            nc.sync.dma_start(wt1s, wt1[:, :])
            nc.sync.dma_start(expTDs, expTD[:, :])
            nc.sync.dma_start(expTTDs, expTTD[:, :])
            nc.sync.dma_start(acss, acs[:, :])
            nc.sync.dma_start(ests, est[:, :])

            # emissions: em128[(b,i), t'*S+s] = sum_h W[i,h] x[b, s*TSEG+t', h]
            HW = GW // 2
            with (
                tc.tile_pool(name="xtiles", bufs=2) as xp,
                tc.tile_pool(name="emps", bufs=4, space="PSUM") as ep,
                tc.tile_pool(name="emsb", bufs=4) as esb,
            ):
                for g in range(NG):
                    x0 = xp.tile([128, GW], fp8, tag="x0")
                    x1 = xp.tile([128, GW], fp8, tag="x1")
                    nc.sync.dma_start(x0, xt[0:128, g * GW:(g + 1) * GW])
                    nc.sync.dma_start(x1, xt[128:256, g * GW:(g + 1) * GW])
                    if not with_emissions:
                        continue
                    for h in range(2):
                        ps = ep.tile([16, HW], f32, tag="ps")
                        es = esb.tile([16, HW], bf16, tag="es")
                        for q in range(HW // 512):
                            sl = slice(q * 512, (q + 1) * 512)
                            xsl = slice(h * HW + q * 512, h * HW + (q + 1) * 512)
                            nc.tensor.matmul(ps[:, sl], wt0s, x0[:, xsl],
                                             start=True, stop=False)
                            nc.tensor.matmul(ps[:, sl], wt1s, x1[:, xsl],
                                             start=False, stop=True)
                        if not with_emcopy:
                            continue
                        if (g * 2 + h) % 2 == 0:
                            nc.scalar.activation(es, ps, COPY)
                        else:
                            nc.vector.tensor_copy(es, ps)
                        nc.sync.dma_start(
                            em128[g * 16:(g + 1) * 16, h * HW:(h + 1) * HW], es)

            if with_emout:
                nc.sync.dma_start(em_out[:, :], em128)
            nq = T // 512
            qorder = []
            lo, hi = 0, nq - 1
            while lo <= hi:
                qorder.append(lo)
                if hi != lo:
                    qorder.append(hi)
                lo += 1
                hi -= 1
            for q in qorder:
                sl = slice(q * 512, (q + 1) * 512)
                nc.scalar.activation(eem[:, sl], em128[:, sl], EXP, bias=cbias)

            # rank-1 segment scan: u = R_s @ 1 (fwd), y = prefix of R_s^T @ 1 (bwd)
            if with_scan:
                with (
                    tc.tile_pool(name="scansb", bufs=3) as ssb,
                    tc.tile_pool(name="scanps", bufs=2, space="PSUM") as sps,
                ):
                    U = ssb.tile([128, S], bf16, tag="U")
                    nc.vector.tensor_scalar_mul(U, eem[:, 0:S], acss)
                    nc.vector.tensor_mul(U[:, 0:1], eem[:, 0:1], ests)
                    YP = sps.tile([128, S], f32, tag="YP")
                    nc.tensor.matmul(YP, expTTDs,
                                     eem[:, (TSEG - 1) * S:TSEG * S],
                                     start=True, stop=True)
                    uy = ssb.tile([128, 2 * S], f32, tag="uy")
                    for k in range(1, TSEG):
                        PS = sps.tile([128, S], f32, tag="PS")
                        nc.tensor.matmul(PS, expTDs, U, start=True, stop=True)
                        if k == TSEG - 1:
                            nc.vector.tensor_mul(uy[:, 0:S], PS,
                                                 eem[:, k * S:(k + 1) * S])
                        else:
                            U2 = ssb.tile([128, S], bf16, tag="U")
                            nc.vector.tensor_mul(U2, PS,
                                                 eem[:, k * S:(k + 1) * S])
                            U = U2
                        tp = TSEG - 1 - k
                        if tp >= 1:
                            ym = ssb.tile([128, S], bf16, tag="ym")
                            nc.vector.tensor_mul(ym, YP,
                                                 eem[:, tp * S:(tp + 1) * S])
                            YP2 = sps.tile([128, S], f32, tag="YP")
                            nc.tensor.matmul(YP2, expTTDs, ym,
                                             start=True, stop=True)
                            YP = YP2
                    nc.vector.tensor_copy(uy[:, S:2 * S], YP)
                    nc.sync.dma_start(uy_out[:, :], uy)

    import concourse.bass as bassmod
    bassmod._bass_rust.generate_event_semaphores(nc)
    return nc


def _run_device(x, W, b, start_transitions, end_transitions, transitions):
    import ml_dtypes
    from concourse.bass_utils import run_bass_kernel_spmd

    nc = _build_nc()

    expT = np.exp(transitions.astype(np.float64)
                  + b.astype(np.float64)[None, :]).astype(np.float32)
    estart_e = np.exp(start_transitions.astype(np.float64)
                      + b.astype(np.float64)).astype(np.float32)
    acs_v = expT.sum(axis=0).astype(np.float32)
    expTD_m = np.zeros((128, 128), np.float32)
    expTTD_m = np.zeros((128, 128), np.float32)
    for bb in range(BS):
        expTD_m[bb * 16:(bb + 1) * 16, bb * 16:(bb + 1) * 16] = expT
        expTTD_m[bb * 16:(bb + 1) * 16, bb * 16:(bb + 1) * 16] = expT.T
    acs128 = np.tile(acs_v, BS)[:, None].astype(np.float32)
    est128 = np.tile(estart_e, BS)[:, None].astype(np.float32)
    cpack_m = np.concatenate([expTD_m, expTTD_m, acs128, est128],
                             axis=1).astype(np.float32)
    wt_full = np.ascontiguousarray(W.T)           # [H, L]
    wtp_m = np.concatenate([wt_full[0:128], wt_full[128:256]],
                           axis=1).astype(ml_dtypes.float8_e4m3)

    in_maps = []
    for c in range(NCORES):
        xs = x[c * BS:(c + 1) * BS]               # [BS, T, H]
        xr = xs.reshape(BS, S, TSEG, H).transpose(3, 0, 2, 1)
        xt_f = np.ascontiguousarray(xr).reshape(H, PTS)      # [256, PTS]
        # pack per group: [128, 2*PTS]: for each g, GW cols of k0 then GW of k1
        x3 = xt_f.reshape(2, 128, NG * 4, 512)
        xt2 = np.ascontiguousarray(
            x3.transpose(1, 2, 0, 3)).reshape(128, 2 * PTS)
        xt_m = xt2.astype(ml_dtypes.float8_e4m3)
        in_maps.append({
            "xt": xt_m, "wtp": wtp_m, "cpack": cpack_m,
        })

    trace = bool(os.environ.get("CRF_KERNEL_TRACE"))
    res = run_bass_kernel_spmd(nc, in_maps, core_ids=list(range(NCORES)),
                               trace=trace)
    global LAST_EXEC_NS
    LAST_EXEC_NS = getattr(res, "exec_time_ns", None)
    results = res.results

    em_parts, denom_parts = [], []
    expT64 = expT.astype(np.float64)
    eend_e = np.exp(end_transitions.astype(np.float64))
    est64 = estart_e.astype(np.float64)
    for c in range(NCORES):
        r = results[c]
        em128 = np.asarray(r["em_out"], dtype=np.float64)     # [128, T]
        UY = np.asarray(r["uy_out"], dtype=np.float64)
        U = UY[:, 0:S].reshape(BS, L, S)
        Y = UY[:, S:2 * S].reshape(BS, L, S)
        e0 = np.exp(em128[:, 0:S] - CSHIFT).reshape(BS, L, S)

        v = np.einsum('ij,bjs->bis', expT64, e0 * Y)
        v[:, :, 0] = e0[:, :, 0] * est64[None, :] * Y[:, :, 0]
        sigma = U.sum(axis=1)                                  # [BS, S]
        d = np.einsum('bis,bis->bs', v[:, :, 1:], U[:, :, 0:S - 1])
        logZ = np.log(np.einsum('l,bl->b', eend_e, U[:, :, S - 1]))
        logZ += np.sum(np.log(d) - np.log(sigma[:, 1:]), axis=1)
        logZ += CSHIFT * T
        denom_parts.append(logZ)

        em = em128.reshape(BS, L, TSEG, S).transpose(0, 3, 2, 1)
        em_parts.append(np.ascontiguousarray(em).reshape(BS, T, L))

    emissions = np.concatenate(em_parts, axis=0)               # [B, T, L] f64
    denom = np.concatenate(denom_parts, axis=0)                # [B]
    return emissions, denom


def _numerator(emissions, start_transitions, end_transitions, transitions,
               tags, mask):
    maskf = mask.astype(np.float64)
    emit_gold = np.take_along_axis(
        emissions, tags[..., None].astype(np.int64), axis=2)[..., 0]
    score = start_transitions[tags[:, 0]].astype(np.float64) + emit_gold[:, 0]
    trans_gold = transitions[tags[:, :-1], tags[:, 1:]].astype(np.float64)
    score = score + np.sum((trans_gold + emit_gold[:, 1:]) * maskf[:, 1:],
                           axis=1)
    seq_ends = np.sum(mask.astype(np.int64), axis=1) - 1
    last_tags = np.take_along_axis(tags.astype(np.int64),
                                   seq_ends[:, None], axis=1)[:, 0]
    return score + end_transitions[last_tags].astype(np.float64)


def _host_denominator(emissions, start_transitions, end_transitions,
                      transitions, mask):
    alpha = start_transitions[None, :] + emissions[:, 0]
    for t in range(1, emissions.shape[1]):
        xm = alpha[:, :, None] + transitions[None, :, :] + \
            emissions[:, t][:, None, :]
        m = np.max(xm, axis=1, keepdims=True)
        nxt = np.squeeze(m, 1) + np.log(np.sum(np.exp(xm - m), axis=1))
        alpha = np.where(mask[:, t][:, None], nxt, alpha)
    xm = alpha + end_transitions[None, :]
    m = np.max(xm, axis=1)
    return m + np.log(np.sum(np.exp(xm - m[:, None]), axis=1))


def kernel(x, W, b, start_transitions, end_transitions, transitions,
           tags, mask):
    x = np.asarray(x, dtype=np.float32)
    W = np.asarray(W, dtype=np.float32)
    b = np.asarray(b, dtype=np.float32)
    start_transitions = np.asarray(start_transitions, dtype=np.float32)
    end_transitions = np.asarray(end_transitions, dtype=np.float32)
    transitions = np.asarray(transitions, dtype=np.float32)
    tags = np.asarray(tags)
    mask = np.asarray(mask).astype(bool)

    use_device = bool(mask.all())   # device scan assumes a full mask
    emissions = None
    if use_device:
        try:
            emissions, denom = _run_device(x, W, b, start_transitions,
                                           end_transitions, transitions)
            emissions = emissions + b.astype(np.float64)[None, None, :]
        except Exception:
            emissions = None
    if emissions is None:
        emissions = (np.einsum('bth,lh->btl', x, W)
                     + b[None, None, :]).astype(np.float64)
        denom = _host_denominator(emissions,
                                  start_transitions.astype(np.float64),
                                  end_transitions.astype(np.float64),
                                  transitions.astype(np.float64), mask)

    score = _numerator(emissions, start_transitions, end_transitions,
                       transitions, tags, mask)
    llh = score - denom
    return np.float32(-np.mean(llh))
